# revision 28
# baseline (speedup 1.0000x reference)
"""Trainium2 Bass kernel for the recurrent actor-critic agent network.

Data-parallel over the env axis B=512 -> 8 cores x 64 envs. Device layout is
fully transposed ([H=128 partitions, rows free]). Key tricks:
  - sigmoid(x) = 0.5*(tanh(x/2)+1): i,f,o gate weights pre-halved on host so
    every gate nonlinearity is Tanh (one ACT table set for the whole scan).
  - h stored as h~=2h, c stored as c~=2c: cell/output updates each become a
    single fused scalar_tensor_tensor DVE op; LN on heads is scale-invariant.
  - done-reset of c folded into the f-gate via a K=1 outer-product matmul
    adding -30*done (sigmoid(f)->0); h-reset is one masked multiply.
  - LN over H (= partition axis) via matmul against an all-1/128 stationary.
  - heads: logits = r*(u - mean*w1) + w0 with u = hs.T @ (g*Wh).T by matmul,
    stats batched 4 row-tiles at a time with step-0 broadcast APs.
"""

import os
import sys

if "/opt/trn_rl_repo" not in sys.path:
    sys.path.append("/opt/trn_rl_repo")

import numpy as np
import ml_dtypes

import concourse.bass as bass
import concourse.tile as tile
from concourse import bacc, mybir
from concourse.bass_utils import run_bass_kernel_spmd

F32 = mybir.dt.float32
BF16 = mybir.dt.bfloat16
AF = mybir.ActivationFunctionType
OP = mybir.AluOpType
AX = mybir.AxisListType

T, B, OBS, ACT, H = 256, 512, 512, 16, 128
NCORE = 8
BL = B // NCORE          # 64 envs per core
R = T * BL               # 16384 rows per core
CH = 512                 # feature/head chunk (rows)
NCH = R // CH            # 32 chunks
MT = R // 128            # 128 head row-tiles
LN_EPS = 1e-5
DONE_KILL = -30.0

bf16np = ml_dtypes.bfloat16


def _bc(ap, reps):
    """View a [P, n] AP as [P, n, reps] by appending a step-0 dim."""
    return bass.AP(ap.tensor, ap.offset, list(ap.ap) + [[0, reps]])


def declare_io(nc):
    d = {}

    def din(name, shape, dt):
        d[name] = nc.dram_tensor(name, list(shape), dt, kind="ExternalInput").ap()

    def dout(name, shape, dt):
        d[name] = nc.dram_tensor(name, list(shape), dt, kind="ExternalOutput").ap()

    din("xT", (OBS, R), BF16)            # x shard, transposed
    din("m1", (128, R), BF16)            # h-mask (1-done), bcast to 128 parts
    din("doneB", (1, R), BF16)           # -30*done row (f-gate kill)
    din("h0T", (128, 2 * BL), BF16)      # 2*h0.T branch-fused [a|c]
    din("c0T", (128, 2 * BL), F32)       # 2*c0.T branch-fused
    din("actF", (128, MT), F32)          # action per row
    din("iota64", (128, 8 * ACT), F32)   # [0..15] tiled 8x
    din("w1B", (128, 8 * ACT), F32)      # sum_h(g*Wh) tiled 8x
    din("w0B", (128, 8 * ACT), F32)      # Wh@ln_b + bh tiled 8x
    din("cVals", (128, 2), F32)          # [w1v, w0v] value-head consts
    for p in ("a", "c"):
        din(p + "W1T", (128, 4, 256), BF16)
        din(p + "b1", (128, 2), F32)
        din(p + "W2T", (128, 2, 128), BF16)
        din(p + "b2", (128, 1), F32)
        din(p + "g1", (128, 1), F32)
        din(p + "bb1", (128, 1), F32)
        din(p + "WihT", (128, 512), BF16)     # i,f,o cols pre-scaled *0.5
        din(p + "WhhT", (128, 512), BF16)     # *0.5 (h~); i,f,o further *0.5
        din(p + "bCols1", (2, 128), BF16)     # [bias_i; bias_f]
        din(p + "bCols2", (2, 128), BF16)     # [bias_g; bias_o]
    din("bRhs", (2, 128), BF16)               # gate-pair indicator rows
    din("headA", (128, 17), BF16)             # [(g*Wh).T | 1/128]
    din("headC", (128, 2), BF16)              # [(g*Wv).T | 1/128]
    din("ones128", (128, 1), BF16)            # 1/128 column

    dout("lpO", (128, MT), F32)
    dout("entO", (128, MT), F32)
    dout("valO", (128, MT), F32)
    dout("hO", (128, 2, BL), BF16)            # final h~ (=2h) branch-fused
    dout("cO", (128, 2 * BL), F32)            # final c~ (=2c) branch-fused
    return d


def build(tc, d):
    from contextlib import ExitStack
    with ExitStack() as _ctx:
        _build(_ctx, tc, d)


def _build(ctx, tc, d):
    nc = tc.nc
    P = ("a", "c")
    PH = os.environ.get("KERNEL_PHASES", "123")

    const = ctx.enter_context(tc.tile_pool(name="const", bufs=1))
    big = ctx.enter_context(tc.tile_pool(name="big", bufs=1))
    xp = ctx.enter_context(tc.tile_pool(name="xp", bufs=2))
    fp = ctx.enter_context(tc.tile_pool(name="fp", bufs=2))
    sp = ctx.enter_context(tc.tile_pool(name="sp", bufs=4))
    hp = ctx.enter_context(tc.tile_pool(name="hp", bufs=2))
    psF = ctx.enter_context(tc.tile_pool(name="psF", bufs=4, space="PSUM"))
    psG = ctx.enter_context(tc.tile_pool(name="psG", bufs=2, space="PSUM"))
    psH = ctx.enter_context(tc.tile_pool(name="psH", bufs=2, space="PSUM"))

    # ---- persistent tiles ----
    fh = big.tile([128, 2, R], BF16)       # feat, overwritten by hs in scan
    m1 = big.tile([128, R], BF16)
    doneB = big.tile([1, R], BF16)
    nc.sync.dma_start(m1[:], d["m1"][:])
    nc.sync.dma_start(doneB[:], d["doneB"][:])

    w = {}
    for p in P:
        for nm, shp in (
            ("W1T", [128, 4, 256]), ("b1", [128, 2]), ("W2T", [128, 2, 128]),
            ("b2", [128, 1]), ("g1", [128, 1]), ("bb1", [128, 1]),
            ("WihT", [128, 512]), ("WhhT", [128, 512]),
            ("bCols1", [2, 128]), ("bCols2", [2, 128]),
        ):
            t = const.tile(shp, d[p + nm].dtype, tag=p + nm)
            nc.sync.dma_start(t[:], d[p + nm][:])
            w[p + nm] = t
    for nm, shp in (
        ("bRhs", [2, 128]), ("headA", [128, 17]), ("headC", [128, 2]),
        ("ones128", [128, 1]), ("actF", [128, MT]), ("iota64", [128, 8 * ACT]),
        ("w1B", [128, 8 * ACT]), ("w0B", [128, 8 * ACT]), ("cVals", [128, 2]),
        ("h0T", [128, 2 * BL]), ("c0T", [128, 2 * BL]),
    ):
        t = const.tile(shp, d[nm].dtype, tag=nm)
        nc.sync.dma_start(t[:], d[nm][:])
        w[nm] = t
    jones = const.tile([128, 128], BF16, tag="jones")
    nc.vector.memset(jones[:], 1.0 / 128.0)
    onecol = const.tile([1, 128], BF16, tag="onecol")
    nc.vector.memset(onecol[:], 1.0)
    epsT = const.tile([128, 1], F32, tag="epsT")
    nc.vector.memset(epsT[:], LN_EPS)

    # ---- output accumulators ----
    lpS = big.tile([128, MT], F32)
    entS = big.tile([128, MT], F32)
    valS = big.tile([128, MT], F32)

    # =============== phase 1: features ===============
    for ch in range(NCH):
        cols = bass.ts(ch, CH)
        xk = xp.tile([128, 4, CH], BF16, tag="xk")
        for k in range(4):
            nc.sync.dma_start(xk[:, k, :], d["xT"][bass.ts(k, 128), cols])
        for p in P:
            pi = 0 if p == "a" else 1
            h1 = fp.tile([128, 2, CH], BF16, tag="h1")
            for mt in range(2):
                ps = psF.tile([128, CH], F32, tag="psF")
                for k in range(4):
                    nc.tensor.matmul(
                        ps[:], w[p + "W1T"][:, k, bass.ts(mt, 128)], xk[:, k, :],
                        start=(k == 0), stop=(k == 3))
                nc.scalar.activation(h1[:, mt, :], ps[:], AF.Relu,
                                     bias=w[p + "b1"][:, mt:mt + 1])
            ps2 = psF.tile([128, CH], F32, tag="psF")
            for k in range(2):
                nc.tensor.matmul(ps2[:], w[p + "W2T"][:, k, :], h1[:, k, :],
                                 start=(k == 0), stop=(k == 1))
            # LN over partitions via J-matmul, two-pass (d = x - mean)
            h2s = fp.tile([128, CH], F32, tag="h2s")
            nc.vector.tensor_scalar(h2s[:], ps2[:], w[p + "b2"][:, 0:1], None,
                                    OP.add)
            h2b = fp.tile([128, CH], BF16, tag="h2b")
            nc.vector.tensor_copy(h2b[:], h2s[:])
            mps = psF.tile([128, CH], F32, tag="psF")
            nc.tensor.matmul(mps[:], jones[:], h2b[:], start=True, stop=True)
            dt_ = fp.tile([128, CH], BF16, tag="dt")
            nc.vector.tensor_tensor(dt_[:], h2s[:], mps[:], OP.subtract)
            sq = fp.tile([128, CH], BF16, tag="sq")
            nc.scalar.activation(sq[:], dt_[:], AF.Square)
            vps = psF.tile([128, CH], F32, tag="psF")
            nc.tensor.matmul(vps[:], jones[:], sq[:], start=True, stop=True)
            sd = fp.tile([128, CH], BF16, tag="sd")
            nc.scalar.activation(sd[:], vps[:], AF.Sqrt, bias=epsT[:])
            rs = fp.tile([128, CH], BF16, tag="rs")
            with nc.allow_low_precision(reason="bf16 rstd is fine at 2e-2 tol"):
                nc.vector.reciprocal(rs[:], sd[:])
            n1 = fp.tile([128, CH], BF16, tag="n1")
            nc.vector.tensor_tensor(n1[:], dt_[:], rs[:], OP.mult)
            nc.scalar.activation(fh[:, pi, cols], n1[:], AF.Relu,
                                 scale=w[p + "g1"][:, 0:1],
                                 bias=w[p + "bb1"][:, 0:1])

    if "2" not in PH:
        return
    tc.no_sync_barrier()

    # =============== phase 2: LSTM scan ===============
    cprev = {}
    for pi in range(2):
        cprev[pi] = sp.tile([128, BL], F32, tag=f"c{pi}", name=f"cprev{pi}")
        nc.vector.tensor_copy(cprev[pi][:], w["c0T"][:, bass.ts(pi, BL)])
    for t in range(T):
        tc64 = bass.ts(t, BL)
        for p in P:
            pi = 0 if p == "a" else 1
            hsrc = (w["h0T"][:, bass.ts(pi, BL)] if t == 0
                    else fh[:, pi, bass.ts(t - 1, BL)])
            hm = sp.tile([128, BL], BF16, tag=f"hm{pi}")
            nc.vector.tensor_tensor(hm[:], hsrc, m1[:, tc64], OP.mult)
            ps = psG.tile([128, 4, BL], F32, tag="psG")
            for g in range(4):
                nc.tensor.matmul(ps[:, g, :], w[p + "WihT"][:, bass.ts(g, 128)],
                                 fh[:, pi, tc64], start=(g == 0), stop=False)
            for g in range(4):
                nc.tensor.matmul(ps[:, g, :], w[p + "WhhT"][:, bass.ts(g, 128)],
                                 hm[:], start=False, stop=False)
            nc.tensor.matmul(ps[:, 0:2, :], w[p + "bCols1"][:], w["bRhs"][:],
                             start=False, stop=False)
            nc.tensor.matmul(ps[:, 2:4, :], w[p + "bCols2"][:], w["bRhs"][:],
                             start=False, stop=False)
            nc.tensor.matmul(ps[:, 1, :], onecol[:], doneB[:, tc64],
                             start=False, stop=True)
            tg = sp.tile([128, 4, BL], BF16, tag=f"tg{pi}")
            nc.scalar.activation(tg[:], ps[:], AF.Tanh)
            a2 = sp.tile([128, BL], F32, tag=f"a2{pi}")
            nc.vector.scalar_tensor_tensor(a2[:], tg[:, 1, :], 1.0,
                                           cprev[pi][:], OP.add, OP.mult)
            bb = sp.tile([128, BL], F32, tag=f"bb{pi}")
            nc.vector.scalar_tensor_tensor(bb[:], tg[:, 0, :], 1.0,
                                           tg[:, 2, :], OP.add, OP.mult)
            cnew = sp.tile([128, BL], F32, tag=f"c{pi}")
            nc.vector.scalar_tensor_tensor(cnew[:], a2[:], 0.5, bb[:], OP.mult,
                                           OP.add)
            tcg = sp.tile([128, BL], BF16, tag=f"tc{pi}")
            nc.scalar.activation(tcg[:], cnew[:], AF.Tanh, scale=0.5)
            nc.vector.scalar_tensor_tensor(fh[:, pi, tc64], tg[:, 3, :], 1.0,
                                           tcg[:], OP.add, OP.mult)
            cprev[pi] = cnew
    for pi in range(2):
        nc.sync.dma_start(d["cO"][:, bass.ts(pi, BL)], cprev[pi][:])
    nc.sync.dma_start(d["hO"][:], fh[:, :, bass.ts(T - 1, BL)])

    if "3" not in PH:
        return
    tc.no_sync_barrier()

    # =============== phase 3: heads ===============
    # pass 1: u/stat matmuls + sqrt (sqrt table set), store per-mt stats
    uS = big.tile([128, MT, ACT], BF16)
    vS = big.tile([128, MT], F32)        # critic u
    stat = big.tile([128, 4, MT], F32)   # a:mean,negr ; c:mean,negr
    for ch in range(NCH):
        sqa = hp.tile([128, CH], BF16, tag="sqa")
        nc.scalar.activation(sqa[:], fh[:, 0, bass.ts(ch, CH)], AF.Square)
        sqc = hp.tile([128, CH], BF16, tag="sqc")
        nc.scalar.activation(sqc[:], fh[:, 1, bass.ts(ch, CH)], AF.Square)
        for mt in range(4):
            j = ch * 4 + mt
            jc = bass.ts(j, 128)
            psB = psH.tile([128, 21], F32, tag="psB")
            psA = psB[:, 0:18]
            psC = psB[:, 18:21]
            nc.tensor.matmul(psB[:, 0:17], fh[:, 0, jc], w["headA"][:],
                             start=True, stop=False)
            nc.tensor.matmul(psB[:, 17:18], sqa[:, bass.ts(mt, 128)],
                             w["ones128"][:], start=False, stop=False)
            nc.tensor.matmul(psB[:, 18:20], fh[:, 1, jc], w["headC"][:],
                             start=False, stop=False)
            nc.tensor.matmul(psB[:, 20:21], sqc[:, bass.ts(mt, 128)],
                             w["ones128"][:], start=False, stop=True)
            nc.vector.tensor_copy(uS[:, j, :], psA[:, 0:16])
            nc.vector.tensor_copy(vS[:, j:j + 1], psC[:, 0:1])
            for base, psm, mcol, ncol in ((0, psA, 16, 17), (2, psC, 1, 2)):
                mean = stat[:, base, j:j + 1]
                nc.vector.tensor_copy(mean, psm[:, mcol:mcol + 1])
                mm = hp.tile([128, 1], F32, tag="mm")
                nc.vector.tensor_tensor(mm[:], mean, mean, OP.mult)
                var = hp.tile([128, 1], F32, tag="var")
                nc.vector.tensor_tensor(var[:], psm[:, ncol:ncol + 1], mm[:],
                                        OP.subtract)
                sd = hp.tile([128, 1], F32, tag="hsd")
                nc.scalar.activation(sd[:], var[:], AF.Sqrt, bias=epsT[:])
                rr = hp.tile([128, 1], F32, tag="rr")
                nc.vector.reciprocal(rr[:], sd[:])
                nc.vector.tensor_scalar(stat[:, base + 1, j:j + 1], rr[:],
                                        -1.0, None, OP.mult)
    # critic value (no transcendentals) — full width in one go
    x1 = hp.tile([128, MT], F32, tag="hx1")
    nc.vector.scalar_tensor_tensor(x1[:], stat[:, 2, :], w["cVals"][:, 0:1],
                                   vS[:], OP.mult, OP.subtract)
    x2 = hp.tile([128, MT], F32, tag="hx2")
    nc.vector.tensor_tensor(x2[:], x1[:], stat[:, 3, :], OP.mult)
    nc.vector.tensor_scalar(valS[:], x2[:], w["cVals"][:, 1:2], None, OP.add)

    tc.no_sync_barrier()

    # pass 2: actor log-softmax. Exp per chunk (exp set); every Ln-dependent
    # op batched full-width after the loop so Ln loads its table ONCE.
    w1B3 = w["w1B"][:].rearrange("p (m a) -> p m a", a=ACT)
    w0B3 = w["w0B"][:].rearrange("p (m a) -> p m a", a=ACT)
    iot3 = w["iota64"][:].rearrange("p (m a) -> p m a", a=ACT)
    ssS = big.tile([128, MT], F32)
    s2S = big.tile([128, MT], F32)
    nmxS = big.tile([128, MT], F32)
    lp1S = big.tile([128, MT], F32)
    G = 8
    for ch in range(MT // G):
        c4 = bass.ts(ch, G)
        u4 = uS[:, c4, :]
        x1 = hp.tile([128, G, ACT], F32, tag="hx1")
        nc.vector.tensor_tensor(x1[:], w1B3, _bc(stat[:, 0, c4], ACT), OP.mult)
        x2 = hp.tile([128, G, ACT], F32, tag="hx2")
        nc.vector.tensor_tensor(x2[:], x1[:], u4, OP.subtract)  # w1*mean - u
        lg = hp.tile([128, G, ACT], F32, tag="lg")
        nc.vector.tensor_tensor(lg[:], x2[:], _bc(stat[:, 1, c4], ACT), OP.mult)
        nc.vector.tensor_tensor(lg[:], lg[:], w0B3, OP.add)
        nc.vector.tensor_reduce(nmxS[:, c4], lg[:], AX.X, OP.max, negate=True)
        dd = hp.tile([128, G, ACT], F32, tag="dd")
        nc.vector.tensor_tensor(dd[:], lg[:], _bc(nmxS[:, c4], ACT), OP.add)
        ee = hp.tile([128, G, ACT], F32, tag="ee")
        nc.scalar.activation(ee[:], dd[:], AF.Exp)
        nc.vector.tensor_reduce(ssS[:, c4], ee[:], AX.X, OP.add)
        oh = hp.tile([128, G, ACT], F32, tag="oh")
        nc.vector.tensor_tensor(oh[:], iot3, _bc(w["actF"][:, c4], ACT),
                                OP.is_equal)
        lps = hp.tile([128, G, ACT], F32, tag="lps")
        nc.vector.tensor_tensor(lps[:], oh[:], lg[:], OP.mult)
        nc.vector.tensor_reduce(lp1S[:, c4], lps[:], AX.X, OP.add)
        el = hp.tile([128, G, ACT], F32, tag="el")
        nc.vector.tensor_tensor(el[:], ee[:], lg[:], OP.mult)
        nc.vector.tensor_reduce(s2S[:, c4], el[:], AX.X, OP.add)
    # full-width tail: one Ln + a handful of [128, MT] DVE ops
    lsT = hp.tile([128, MT], F32, tag="sqa")
    nc.scalar.activation(lsT[:], ssS[:], AF.Ln)
    lseT = hp.tile([128, MT], F32, tag="sqc")
    nc.vector.tensor_tensor(lseT[:], lsT[:], nmxS[:], OP.subtract)
    nc.vector.tensor_tensor(lpS[:], lp1S[:], lseT[:], OP.subtract)
    rsT = hp.tile([128, MT], F32, tag="oh")
    nc.vector.reciprocal(rsT[:], ssS[:])
    qT = hp.tile([128, MT], F32, tag="lg")
    nc.vector.tensor_tensor(qT[:], s2S[:], rsT[:], OP.mult)
    nc.vector.tensor_tensor(entS[:], lseT[:], qT[:], OP.subtract)

    nc.sync.dma_start(d["lpO"][:], lpS[:])
    nc.sync.dma_start(d["entO"][:], entS[:])
    nc.sync.dma_start(d["valO"][:], valS[:])


def _flat2(tg, g):
    return tg[:, g, :, :].rearrange("p b e -> p (b e)")


def _flat3(t3):
    return t3[:].rearrange("p b e -> p (b e)")


# ===================== host side =====================

def _prep_core(inp, c):
    lo, hi = c * BL, (c + 1) * BL
    bf = lambda a: np.ascontiguousarray(a).astype(bf16np)
    f = lambda a: np.ascontiguousarray(a).astype(np.float32)

    x3 = inp["x"].reshape(T, B, OBS)[:, lo:hi]            # [T,64,OBS]
    xT = bf(x3.reshape(R, OBS).T)                         # [OBS,R]
    done = inp["done"].reshape(T, B)[:, lo:hi].astype(np.float32)  # [T,64]
    m1row = (1.0 - done).reshape(R)
    m1 = bf(np.broadcast_to(m1row, (128, R)))
    doneB = bf((DONE_KILL * done).reshape(1, R))
    act = inp["action"].reshape(T, B)[:, lo:hi].reshape(R)
    actF = f(act.reshape(MT, 128).T)
    iota = np.tile(np.arange(ACT, dtype=np.float32), 8)
    iota64 = f(np.broadcast_to(iota, (128, 8 * ACT)))

    d = {
        "xT": xT, "m1": m1, "doneB": doneB, "actF": actF, "iota64": iota64,
        "h0T": bf(np.concatenate([2 * inp["actor_h0"][lo:hi].T,
                                  2 * inp["critic_h0"][lo:hi].T], 1)),
        "c0T": f(np.concatenate([2 * inp["actor_c0"][lo:hi].T,
                                 2 * inp["critic_c0"][lo:hi].T], 1)),
        "bRhs": bf(np.kron(np.eye(2, dtype=np.float32), np.ones((1, 64)))),
        "ones128": bf(np.full((128, 1), 1.0 / 128.0)),
    }
    for p in ("a", "c"):
        W1, b1 = inp[p + "_W1"], inp[p + "_b1"]
        W2, b2 = inp[p + "_W2"], inp[p + "_b2"]
        d[p + "W1T"] = bf(W1.T.reshape(4, 128, 256).transpose(1, 0, 2))
        d[p + "b1"] = f(b1.reshape(2, 128).T)
        d[p + "W2T"] = bf(W2.T.reshape(2, 128, 128).transpose(1, 0, 2))
        d[p + "b2"] = f(b2.reshape(128, 1))
        d[p + "g1"] = f(inp[p + "_ln1_g"].reshape(128, 1))
        d[p + "bb1"] = f(inp[p + "_ln1_b"].reshape(128, 1))
        gs = np.array([0.5, 0.5, 1.0, 0.5], np.float32).repeat(H)  # i,f,g,o
        d[p + "WihT"] = bf((inp[p + "_Wih"] * gs[:, None]).T)
        d[p + "WhhT"] = bf((inp[p + "_Whh"] * (0.5 * gs)[:, None]).T)
        bias = (inp[p + "_bih"] + inp[p + "_bhh"]) * gs
        d[p + "bCols1"] = bf(bias.reshape(4, 128)[0:2])
        d[p + "bCols2"] = bf(bias.reshape(4, 128)[2:4])
    g2, b2l = inp["a_ln2_g"], inp["a_ln2_b"]
    Wh, bh = inp["a_Wh"], inp["a_bh"]
    d["headA"] = bf(np.concatenate([(g2[None, :] * Wh).T,
                                    np.full((128, 1), 1.0 / 128.0)], 1))
    w1 = (g2[None, :] * Wh).sum(1)                        # [16]
    w0 = Wh @ b2l + bh
    d["w1B"] = f(np.broadcast_to(np.tile(w1, 8), (128, 8 * ACT)))
    d["w0B"] = f(np.broadcast_to(np.tile(w0, 8), (128, 8 * ACT)))
    g2c, b2c = inp["c_ln2_g"], inp["c_ln2_b"]
    Wv, bv = inp["c_Wh"], inp["c_bh"]
    d["headC"] = bf(np.concatenate([(g2c[None, :] * Wv).T,
                                    np.full((128, 1), 1.0 / 128.0)], 1))
    w1v = float((g2c * Wv[0]).sum())
    w0v = float(Wv[0] @ b2c + bv[0])
    d["cVals"] = f(np.broadcast_to(np.array([w1v, w0v], np.float32), (128, 2)))
    return d


_PROG = None
last_exec_time_ns = None


def _get_prog():
    global _PROG
    if _PROG is None:
        nc = bacc.Bacc("TRN2", target_bir_lowering=False, debug=False)
        d = declare_io(nc)
        with tile.TileContext(nc) as tc:
            build(tc, d)
        nc.compile()
        _PROG = nc
    return _PROG


def kernel(**inputs):
    global last_exec_time_ns
    inputs = {k: np.asarray(v) for k, v in inputs.items()}
    nc = _get_prog()
    in_maps = [_prep_core(inputs, c) for c in range(NCORE)]
    trace = bool(int(os.environ.get("BASS_KERNEL_TRACE", "0")))
    res = run_bass_kernel_spmd(nc, in_maps, core_ids=list(range(NCORE)),
                               trace=trace)
    last_exec_time_ns = res.exec_time_ns
    lp = np.empty((T, B), np.float32)
    ent = np.empty((T, B), np.float32)
    val = np.empty((T, B), np.float32)
    ahT = np.empty((B, H), np.float32)
    acT = np.empty((B, H), np.float32)
    chT = np.empty((B, H), np.float32)
    ccT = np.empty((B, H), np.float32)
    for c in range(NCORE):
        r = res.results[c]
        lo, hi = c * BL, (c + 1) * BL
        lp[:, lo:hi] = r["lpO"].astype(np.float32).T.reshape(T, BL)
        ent[:, lo:hi] = r["entO"].astype(np.float32).T.reshape(T, BL)
        val[:, lo:hi] = r["valO"].astype(np.float32).T.reshape(T, BL)
        hO = np.asarray(r["hO"]).astype(np.float32) * 0.5
        cO = np.asarray(r["cO"]).astype(np.float32) * 0.5
        ahT[lo:hi] = hO[:, 0, :].T
        chT[lo:hi] = hO[:, 1, :].T
        acT[lo:hi] = cO[:, 0:BL].T
        ccT[lo:hi] = cO[:, BL:].T
    return (inputs["action"].astype(np.int32), lp.reshape(-1),
            ent.reshape(-1), val.reshape(-1, 1), ahT, acT, chT, ccT)
